# revision 33
# baseline (speedup 1.0000x reference)
"""Causal multi-head attention (B=2, S=2048, D=1024, H=16) on 8 trn2 NeuronCores.

Sharding: data-parallel over batch (2 groups of 4 cores), tensor-parallel over
heads within a group (4 heads/core). Each core computes qkv projection for its
head slice, causal flash-style attention, and a partial output projection;
the host sums the 4 partials per batch element.

Device-side layout notes (per core):
  - x arrives pre-transposed (host does x[b].T, cast bf16): xT [D=1024, S=2048].
  - q,k are produced transposed (qT/kT [head_dim, S]) so score matmuls
    contract head_dim on partitions: scores^T [Sk, Sq] = kT_tile^T @ qT.
    Head pairs sit at partition bases 0/64, so their K=64 score matmuls run
    concurrently in distinct PE row groups.
  - softmax denominators come from a ones-column appended to V (M=65 in the
    PV matmul); 1/l is one batched DVE reciprocal per head pair, broadcast
    across partitions by a DRAM-bounce DMA. Nothing lands on the PE queue.
  - causal masking zeroes the k>q triangle of exp(scores) on GPSIMD; exp
    needs no max-subtraction (|score| < ~4). Diagonal tiles restrict all
    work (matmul N, exp, mask) to the valid column range.
  - everything is emitted interleaved per q-chunk (projection c, attention c,
    output-projection c-1) so ACT exp work overlaps PE projection work.
"""

import numpy as np
import ml_dtypes

import concourse.bass as bass
import concourse.mybir as mybir
import concourse.tile as tile
from concourse import bacc
from concourse.bass_utils import run_bass_kernel_spmd

B, S, D, H = 2, 2048, 1024, 16
DH = D // H              # 64
HL = 4                   # heads per core
N_CORES = 8
KO = D // 128            # 8 contraction subtiles for the qkv projection
CH = 512                 # q chunk (matmul moving dim)
NCH = S // CH            # 4
KT = S // 128            # 16 k tiles
VW = DH + 1              # v columns incl the ones column
F32 = mybir.dt.float32
BF16 = mybir.dt.bfloat16
EXP = mybir.ActivationFunctionType.Exp

_cached = {}


def build():
    if "nc" in _cached:
        return _cached["nc"]

    nc = bacc.Bacc("TRN2", target_bir_lowering=False, debug=False)

    xT = nc.dram_tensor("xT", [D, S], BF16, kind="ExternalInput")
    wq = nc.dram_tensor("wq", [D, HL * DH], BF16, kind="ExternalInput")
    wk = nc.dram_tensor("wk", [D, HL * DH], BF16, kind="ExternalInput")
    wv = nc.dram_tensor("wv", [D, HL * DH], BF16, kind="ExternalInput")
    wo = nc.dram_tensor("wo", [HL * DH, D], BF16, kind="ExternalInput")
    out = nc.dram_tensor("out", [S, D], F32, kind="ExternalOutput")

    xT_v = xT[:].rearrange("(ko p) s -> p ko s", p=128)      # [128, 8, 2048]
    wq_v = wq[:].rearrange("(ko p) m -> p ko m", p=128)      # [128, 8, 256]
    wk_v = wk[:].rearrange("(ko p) m -> p ko m", p=128)
    wv_v = wv[:].rearrange("(ko p) m -> p ko m", p=128)
    wo_v = wo[:].rearrange("(ko p) n -> p ko n", p=128)      # [128, 2, 1024]
    out_v = out[:].rearrange("(t p) n -> p t n", p=128)      # [128, 16, 1024]

    with tile.TileContext(nc) as tc:
        with (
            tc.tile_pool(name="persist", bufs=1) as pp,
            tc.tile_pool(name="mm", bufs=2, space=bass.MemorySpace.PSUM) as mmp,
            tc.tile_pool(name="ps2s", bufs=2, space=bass.MemorySpace.PSUM) as ps2s,
            tc.tile_pool(name="ps2o", bufs=1, space=bass.MemorySpace.PSUM) as ps2o,
            tc.tile_pool(name="ptp", bufs=3) as ptp,
            tc.tile_pool(name="stg", bufs=3) as stg,
            tc.tile_pool(name="ostg", bufs=4) as ostg,
            tc.tile_pool(name="dstg", bufs=4, space="DRAM") as dstg,
            tc.tile_pool(name="tkp", bufs=1) as tkp,
        ):
            # per-chunk tiles so later phases start as soon as inputs land
            xT_sb = [pp.tile([128, KO, CH], BF16, tag=f"xT{c}", name=f"xT{c}")
                     for c in range(NCH)]
            wq_sb = pp.tile([128, KO, HL * DH], BF16, tag="wq")
            wk_sb = pp.tile([128, KO, HL * DH], BF16, tag="wk")
            wv_sb = pp.tile([128, KO, HL * DH], BF16, tag="wv")
            wo_sb = pp.tile([128, 2, D], BF16, tag="wo")
            qT_sb = [[pp.tile([128, CH], BF16, tag=f"qT{m}{c}", name=f"qT{m}{c}")
                      for c in range(NCH)] for m in range(2)]
            kT_sb = [[pp.tile([128, CH], BF16, tag=f"kT{m}{c}", name=f"kT{m}{c}")
                      for c in range(NCH)] for m in range(2)]
            # v with a ones column appended per head: [v_h (64) | 1]
            v_sb = [pp.tile([128, HL * VW], BF16, tag=f"v{t}", name=f"v{t}")
                    for t in range(KT)]
            # normalized attention output, transposed: proj lhsT, per q-chunk
            pj_sb = [[pp.tile([128, CH], BF16, tag=f"pj{c}{p}", name=f"pj{c}{p}")
                      for p in range(2)] for c in range(NCH)]

            # first matmul needs wk + xT[0]: load those first
            nc.scalar.dma_start(wk_sb[:], wk_v)
            nc.sync.dma_start(xT_sb[0][:, 0:4, :], xT_v[:, 0:4, 0:CH])
            nc.scalar.dma_start(xT_sb[0][:, 4:8, :], xT_v[:, 4:8, 0:CH])
            nc.scalar.dma_start(wq_sb[:], wq_v)
            nc.scalar.dma_start(wv_sb[:], wv_v)
            nc.scalar.dma_start(wo_sb[:], wo_v)
            for c in range(1, NCH):
                nc.sync.dma_start(xT_sb[c][:], xT_v[:, :, c * CH:(c + 1) * CH])
            bias0 = pp.tile([128, 1], F32, tag="bias0")
            nc.gpsimd.memset(bias0[:], 0.0)
            for t in range(KT):
                nc.gpsimd.memset(v_sb[t][:], 1.0)

            def p1_groups(c):
                """qT, kT, v projection psum-groups for chunk c (injectable)."""
                groups = []
                for wsb, dst in ((wk_sb, kT_sb), (wq_sb, qT_sb)):
                    for m in range(2):          # 128-col tiles (2 heads each)
                        def g(wsb=wsb, dst=dst, m=m):
                            ps = mmp.tile([128, CH], F32, tag="mm", name="mm")
                            for ko in range(KO):
                                nc.tensor.matmul(
                                    ps,
                                    lhsT=wsb[:, ko, m * 128:(m + 1) * 128],
                                    rhs=xT_sb[c][:, ko, :],
                                    start=(ko == 0),
                                    stop=(ko == KO - 1),
                                )
                            nc.vector.tensor_copy(dst[m][c][:], ps)
                        groups.append(g)
                for tt in range(4):             # v tiles of this chunk
                    def g(tt=tt):
                        t = 4 * c + tt
                        ps = mmp.tile([128, CH], F32, tag="mm", name="mm")
                        for ko in range(KO):
                            nc.tensor.matmul(
                                ps[:, :HL * DH],
                                lhsT=xT_sb[c][:, ko, tt * 128:(tt + 1) * 128],
                                rhs=wv_sb[:, ko, :],
                                start=(ko == 0),
                                stop=(ko == KO - 1),
                            )
                        dst = v_sb[t][:].rearrange("p (h e) -> p h e", e=VW)
                        src = ps[:, :HL * DH].rearrange("p (h e) -> p h e", e=DH)
                        nc.vector.tensor_copy(dst[:, :, :DH], src)
                    groups.append(g)
                return groups

            def attention(c, pr, inject=()):
                """Causal attention for q-chunk c, head pair pr (2pr, 2pr+1).
                Emitters from `inject` are spread between t-steps so their PE
                work fills the gaps of this ACT-bound stretch. The pair's
                denominator rows are DMA'd dense onto partitions 0/1 of its
                own lb2 tile (so a pair can be finished independently)."""
                lb2 = stg.tile([2, CH], F32, tag=f"lb2{pr}", name=f"lb2{pr}")
                inject = list(inject)
                nk = 4 * c + 4                  # k tiles this chunk needs
                every = max(1, (nk + len(inject) - 1) // max(1, len(inject))) \
                    if inject else 0
                po = [ps2o.tile([65, CH], F32, tag=f"po{hh}", name=f"po{hh}")
                      for hh in range(2)]

                def emit_score(t):
                    """score pair matmuls + exp + causal triangle mask."""
                    d = t - 4 * c               # >= 0 on diagonal tiles
                    lo = 128 * max(d, 0)        # first valid column in chunk
                    st = ps2s.tile([128, 2, CH], F32, tag="s", name="s")
                    for hh in range(2):
                        nc.tensor.matmul(
                            st[:, hh, lo:],
                            lhsT=kT_sb[pr][t // 4][
                                hh * 64:hh * 64 + 64,
                                (t % 4) * 128:(t % 4) * 128 + 128],
                            rhs=qT_sb[pr][c][hh * 64:hh * 64 + 64, lo:],
                            start=True,
                            stop=True,
                        )
                    pt = ptp.tile([128, 2, CH], BF16, tag="pt", name="pt")
                    nc.scalar.activation(
                        pt[:, :, lo:], st[:, :, lo:], EXP,
                        bias=bias0[:, 0:1],
                        scale=float(DH) ** -0.5,
                    )
                    if d >= 0:                  # zero the k>q triangle, which
                        # only spans the first 128 columns of the valid range
                        nc.gpsimd.affine_select(
                            out=pt[:, :, lo:lo + 128],
                            in_=pt[:, :, lo:lo + 128],
                            compare_op=mybir.AluOpType.is_ge,
                            fill=0.0,
                            base=0,
                            pattern=[[0, 2], [1, 128]],
                            channel_multiplier=-1,
                        )
                    return pt, lo

                # software pipeline: scores run one t ahead of the PV matmuls
                # so the PE never sits behind the exp of the tile it consumes
                pts = {0: emit_score(0)}
                for t in range(nk):
                    if t + 1 < nk:
                        pts[t + 1] = emit_score(t + 1)
                    pt, lo = pts.pop(t)
                    for hh in range(2):
                        h = 2 * pr + hh
                        nc.tensor.matmul(
                            po[hh][:, lo:],
                            lhsT=v_sb[t][:, h * VW:(h + 1) * VW],
                            rhs=pt[:, hh, lo:],
                            start=(t == 0),
                            stop=(t == nk - 1),
                        )
                    if inject and t % every == every - 1:
                        inject.pop(0)()
                for g in inject:
                    g()
                # chain head: stage po to SBUF, move l rows dense onto
                # partitions 2pr/2pr+1 of the chunk's lb4 (SBUF->SBUF DMA
                # shifts partitions; compute engines cannot)
                ots = []
                for hh in range(2):
                    ot = stg.tile([65, CH], F32, tag=f"ot{pr}{hh}",
                                  name=f"ot{pr}{hh}")
                    nc.vector.tensor_copy(ot[64:65], po[hh][64:65])
                    nc.sync.dma_start(lb2[hh:hh + 1], ot[64:65])
                    nc.vector.tensor_copy(ot[0:64], po[hh][0:64])
                    ots.append(ot)
                return ots, lb2

            def fin_all(c, r0, r1):
                """batched finish of both pairs: one reciprocal over all 4
                denominator rows (gathered by two cheap SBUF->SBUF DMAs)."""
                lb4 = stg.tile([4, CH], F32, tag="lb4", name="lb4")
                nc.sync.dma_start(lb4[0:2], r0[1][:])
                nc.sync.dma_start(lb4[2:4], r1[1][:])
                rb = stg.tile([4, CH], F32, tag="rb4", name="rb4")
                nc.vector.reciprocal(rb, lb4)
                sr4 = dstg.tile([4, CH], F32, tag="sr4", name="sr4")
                nc.sync.dma_start(sr4[:], rb[:])
                for pr, rr in ((0, r0), (1, r1)):
                    for hh in (1, 0):
                        idx = 2 * pr + hh
                        rep = stg.tile([64, CH], F32, tag=f"rep{pr}{hh}",
                                       name=f"rep{pr}{hh}")
                        nc.sync.dma_start(
                            rep[:], sr4[idx:idx + 1].to_broadcast((64, CH)))
                        if hh == 0:
                            nc.vector.tensor_mul(
                                pj_sb[c][pr][0:64, :], rr[0][hh][0:64], rep[:])
                        else:
                            tmp = stg.tile([64, CH], BF16, tag=f"tmp{pr}",
                                           name=f"tmp{pr}")
                            nc.vector.tensor_mul(tmp, rr[0][hh][0:64], rep[:])
                            nc.sync.dma_start(pj_sb[c][pr][64:128, :], tmp)

            def fin_pair(c, pr, ots, lb2):
                """reciprocal of one pair's denominators + scale into pj.
                Returns the chain tiles (used as warm-up pegs at the tail)."""
                rb = stg.tile([2, CH], F32, tag=f"rb{pr}", name=f"rb{pr}")
                nc.vector.reciprocal(rb, lb2[:])
                sr2 = dstg.tile([2, CH], F32, tag=f"sr{pr}", name=f"sr{pr}")
                nc.sync.dma_start(sr2[:], rb[:])
                reps = []
                # odd head (needs the partition-shift DMA) first, so the
                # final dependency of pj is the cheap direct multiply
                for hh in (1, 0):
                    rep = stg.tile([64, CH], F32, tag=f"rep{pr}{hh}",
                                   name=f"rep{pr}{hh}")
                    nc.sync.dma_start(
                        rep[:], sr2[hh:hh + 1].to_broadcast((64, CH)))
                    reps.append(rep)
                    if hh == 0:
                        nc.vector.tensor_mul(
                            pj_sb[c][pr][0:64, :], ots[hh][0:64], rep[:])
                    else:
                        tmp = stg.tile([64, CH], BF16, tag=f"tmp{pr}",
                                       name=f"tmp{pr}")
                        nc.vector.tensor_mul(tmp, ots[hh][0:64], rep[:])
                        nc.sync.dma_start(pj_sb[c][pr][64:128, :], tmp)
                return rb, reps

            def p3_groups(c, ko_list=(0, 1), accum=False, tail=False):
                """partial output projection psum-groups for chunk c over the
                given pj pairs; accum=True adds into DRAM instead of writing
                (used to split the last chunk's projection per pair)."""
                groups = []
                for tt in range(4):
                    for n2 in range(2):
                        def g(tt=tt, n2=n2):
                            mt = 4 * c + tt
                            prt = mmp.tile([128, CH], F32, tag="mm", name="mm")
                            for j, ko in enumerate(ko_list):
                                nc.tensor.matmul(
                                    prt,
                                    lhsT=pj_sb[c][ko][:, tt * 128:tt * 128 + 128],
                                    rhs=wo_sb[:, ko, n2 * CH:(n2 + 1) * CH],
                                    start=(j == 0),
                                    stop=(j == len(ko_list) - 1),
                                )
                            ob = ostg.tile([128, CH], F32, tag="ob", name="ob")
                            if tail and n2 == 0:
                                nc.scalar.copy(ob, prt)
                            else:
                                nc.vector.tensor_copy(ob, prt)
                            nc.scalar.dma_start(
                                out_v[:, mt, n2 * CH:(n2 + 1) * CH], ob,
                                accum_op=(mybir.AluOpType.add if accum
                                          else mybir.AluOpType.bypass))
                        groups.append(g)
                return groups

            pending = {}
            tks = {}
            lc = NCH - 1

            def prepass_groups():
                """ko=0 (pair 0) half of the last chunk's projection, stashed
                in SBUF tk tiles; runs inside the last attention block."""
                groups = []
                for tt in range(4):
                    for n2 in range(2):
                        def g(tt=tt, n2=n2):
                            prt = mmp.tile([128, CH], F32, tag="mm", name="mm")
                            nc.tensor.matmul(
                                prt,
                                lhsT=pj_sb[lc][0][:, tt * 128:tt * 128 + 128],
                                rhs=wo_sb[:, 0, n2 * CH:(n2 + 1) * CH],
                                start=True, stop=True,
                            )
                            tk = tkp.tile([128, CH], F32, tag=f"tk{tt}{n2}",
                                          name=f"tk{tt}{n2}")
                            nc.vector.tensor_copy(tk, prt)
                            tks[(tt, n2)] = tk
                        groups.append(g)
                return groups

            for g in p1_groups(0):
                g()
            for c in range(NCH):
                inj0, inj1 = [], []
                if c > 0:
                    inj0.append(lambda c=c: fin_all(
                        c - 1, pending[(c - 1, 0)], pending[(c - 1, 1)]))
                if c + 1 < NCH:
                    p1 = p1_groups(c + 1)
                    inj0 += p1[:4]
                    inj1 += p1[4:]
                if c > 0:
                    inj1 += p3_groups(c - 1)
                r0 = attention(c, 0, inj0)
                if c == lc:
                    inj1.insert(0, lambda: fin_pair(lc, 0, *r0))
                    inj1 += prepass_groups()
                r1 = attention(c, 1, inj1)
                pending[(c, 0)] = r0
                pending[(c, 1)] = r1

            # ---- tail: finish pair 1, keep the PE warm through the chain
            # with dummy matmuls pegged to chain outputs, then the ko=1 half
            # of the projection + add of the stashed ko=0 half
            ots1, lb2_1 = pending[(lc, 1)]
            rb, reps = fin_pair(lc, 1, *pending[(lc, 1)])
            sdummy = ps2s.tile([128, 2, CH], F32, tag="s", name="s")
            for peg in (ots1[0], ots1[1], lb2_1, rb, reps[0], reps[1]):
                pk = min(peg.shape[0], 64)
                nc.tensor.matmul(
                    sdummy[:, 0, :],
                    lhsT=peg[0:pk, 0:128], rhs=peg[0:pk, :],
                    start=True, stop=True,
                )
            for tt in range(4):
                for n2 in range(2):
                    prt = mmp.tile([128, CH], F32, tag="mm", name="mm")
                    nc.tensor.matmul(
                        prt,
                        lhsT=pj_sb[lc][1][:, tt * 128:tt * 128 + 128],
                        rhs=wo_sb[:, 1, n2 * CH:(n2 + 1) * CH],
                        start=True, stop=True,
                    )
                    ob = ostg.tile([128, CH], F32, tag="ob", name="ob")
                    nc.vector.tensor_add(ob, prt, tks[(tt, n2)][:])
                    eng = nc.scalar if (2 * tt + n2) % 2 == 0 else nc.sync
                    eng.dma_start(
                        out_v[:, 4 * lc + tt, n2 * CH:(n2 + 1) * CH], ob)

    nc.compile()
    _cached["nc"] = nc
    return nc


def make_in_maps(x, w_qkv, w_out):
    bf = ml_dtypes.bfloat16
    in_maps = []
    for core in range(N_CORES):
        b, h0 = core // 4, (core % 4) * HL
        c0 = h0 * DH
        in_maps.append({
            "xT": np.ascontiguousarray(x[b].T).astype(bf),
            "wq": w_qkv[:, c0:c0 + HL * DH].astype(bf),
            "wk": w_qkv[:, D + c0:D + c0 + HL * DH].astype(bf),
            "wv": w_qkv[:, 2 * D + c0:2 * D + c0 + HL * DH].astype(bf),
            "wo": w_out[c0:c0 + HL * DH, :].astype(bf),
        })
    return in_maps


def run_sharded(x, w_qkv, w_out, trace=False):
    nc = build()
    res = run_bass_kernel_spmd(
        nc, make_in_maps(x, w_qkv, w_out), core_ids=list(range(N_CORES)),
        trace=trace,
    )
    out = np.zeros((B, S, D), np.float32)
    for core in range(N_CORES):
        out[core // 4] += res.results[core]["out"]
    return out, res.exec_time_ns


def kernel(x, w_qkv, w_out):
    out, _ = run_sharded(x, w_qkv, w_out)
    return out


# revision 34
# speedup vs baseline: 1.0029x; 1.0029x over previous
"""Causal multi-head attention (B=2, S=2048, D=1024, H=16) on 8 trn2 NeuronCores.

Sharding: data-parallel over batch (2 groups of 4 cores), tensor-parallel over
heads within a group (4 heads/core). Each core computes qkv projection for its
head slice, causal flash-style attention, and a partial output projection;
the host sums the 4 partials per batch element.

Device-side layout notes (per core):
  - x arrives pre-transposed (host does x[b].T, cast bf16): xT [D=1024, S=2048].
  - q,k are produced transposed (qT/kT [head_dim, S]) so score matmuls
    contract head_dim on partitions: scores^T [Sk, Sq] = kT_tile^T @ qT.
    Head pairs sit at partition bases 0/64, so their K=64 score matmuls run
    concurrently in distinct PE row groups.
  - softmax denominators come from a ones-column appended to V (M=65 in the
    PV matmul); 1/l is one batched DVE reciprocal per head pair, broadcast
    across partitions by a DRAM-bounce DMA. Nothing lands on the PE queue.
  - causal masking zeroes the k>q triangle of exp(scores) on GPSIMD; exp
    needs no max-subtraction (|score| < ~4). Diagonal tiles restrict all
    work (matmul N, exp, mask) to the valid column range.
  - everything is emitted interleaved per q-chunk (projection c, attention c,
    output-projection c-1) so ACT exp work overlaps PE projection work.
"""

import numpy as np
import ml_dtypes

import concourse.bass as bass
import concourse.mybir as mybir
import concourse.tile as tile
from concourse import bacc
from concourse.bass_utils import run_bass_kernel_spmd

B, S, D, H = 2, 2048, 1024, 16
DH = D // H              # 64
HL = 4                   # heads per core
N_CORES = 8
KO = D // 128            # 8 contraction subtiles for the qkv projection
CH = 512                 # q chunk (matmul moving dim)
NCH = S // CH            # 4
KT = S // 128            # 16 k tiles
VW = DH + 1              # v columns incl the ones column
F32 = mybir.dt.float32
BF16 = mybir.dt.bfloat16
EXP = mybir.ActivationFunctionType.Exp

_cached = {}


def build():
    if "nc" in _cached:
        return _cached["nc"]

    nc = bacc.Bacc("TRN2", target_bir_lowering=False, debug=False)

    xT = nc.dram_tensor("xT", [D, S], BF16, kind="ExternalInput")
    wq = nc.dram_tensor("wq", [D, HL * DH], BF16, kind="ExternalInput")
    wk = nc.dram_tensor("wk", [D, HL * DH], BF16, kind="ExternalInput")
    wv = nc.dram_tensor("wv", [D, HL * DH], BF16, kind="ExternalInput")
    wo = nc.dram_tensor("wo", [HL * DH, D], BF16, kind="ExternalInput")
    out = nc.dram_tensor("out", [S, D], F32, kind="ExternalOutput")

    xT_v = xT[:].rearrange("(ko p) s -> p ko s", p=128)      # [128, 8, 2048]
    wq_v = wq[:].rearrange("(ko p) m -> p ko m", p=128)      # [128, 8, 256]
    wk_v = wk[:].rearrange("(ko p) m -> p ko m", p=128)
    wv_v = wv[:].rearrange("(ko p) m -> p ko m", p=128)
    wo_v = wo[:].rearrange("(ko p) n -> p ko n", p=128)      # [128, 2, 1024]
    out_v = out[:].rearrange("(t p) n -> p t n", p=128)      # [128, 16, 1024]

    with tile.TileContext(nc) as tc:
        with (
            tc.tile_pool(name="persist", bufs=1) as pp,
            tc.tile_pool(name="mm", bufs=2, space=bass.MemorySpace.PSUM) as mmp,
            tc.tile_pool(name="ps2s", bufs=2, space=bass.MemorySpace.PSUM) as ps2s,
            tc.tile_pool(name="ps2o", bufs=1, space=bass.MemorySpace.PSUM) as ps2o,
            tc.tile_pool(name="ptp", bufs=3) as ptp,
            tc.tile_pool(name="stg", bufs=3) as stg,
            tc.tile_pool(name="ostg", bufs=4) as ostg,
            tc.tile_pool(name="dstg", bufs=4, space="DRAM") as dstg,
            tc.tile_pool(name="tkp", bufs=1) as tkp,
        ):
            # per-chunk tiles so later phases start as soon as inputs land
            xT_sb = [pp.tile([128, KO, CH], BF16, tag=f"xT{c}", name=f"xT{c}")
                     for c in range(NCH)]
            wq_sb = pp.tile([128, KO, HL * DH], BF16, tag="wq")
            wk_sb = pp.tile([128, KO, HL * DH], BF16, tag="wk")
            wv_sb = pp.tile([128, KO, HL * DH], BF16, tag="wv")
            wo_sb = pp.tile([128, 2, D], BF16, tag="wo")
            qT_sb = [[pp.tile([128, CH], BF16, tag=f"qT{m}{c}", name=f"qT{m}{c}")
                      for c in range(NCH)] for m in range(2)]
            kT_sb = [[pp.tile([128, CH], BF16, tag=f"kT{m}{c}", name=f"kT{m}{c}")
                      for c in range(NCH)] for m in range(2)]
            # v with a ones column appended per head: [v_h (64) | 1]
            v_sb = [pp.tile([128, HL * VW], BF16, tag=f"v{t}", name=f"v{t}")
                    for t in range(KT)]
            # normalized attention output, transposed: proj lhsT, per q-chunk
            pj_sb = [[pp.tile([128, CH], BF16, tag=f"pj{c}{p}", name=f"pj{c}{p}")
                      for p in range(2)] for c in range(NCH)]

            # first matmul needs wk + xT[0]: load those first
            nc.scalar.dma_start(wk_sb[:], wk_v)
            nc.sync.dma_start(xT_sb[0][:, 0:4, :], xT_v[:, 0:4, 0:CH])
            nc.scalar.dma_start(xT_sb[0][:, 4:8, :], xT_v[:, 4:8, 0:CH])
            nc.scalar.dma_start(wq_sb[:], wq_v)
            nc.scalar.dma_start(wv_sb[:], wv_v)
            nc.scalar.dma_start(wo_sb[:], wo_v)
            for c in range(1, NCH):
                nc.sync.dma_start(xT_sb[c][:], xT_v[:, :, c * CH:(c + 1) * CH])
            bias0 = pp.tile([128, 1], F32, tag="bias0")
            nc.gpsimd.memset(bias0[:], 0.0)
            for t in range(KT):
                nc.gpsimd.memset(v_sb[t][:], 1.0)

            def p1_groups(c):
                """qT, kT, v projection psum-groups for chunk c (injectable)."""
                groups = []
                for wsb, dst in ((wk_sb, kT_sb), (wq_sb, qT_sb)):
                    for m in range(2):          # 128-col tiles (2 heads each)
                        def g(wsb=wsb, dst=dst, m=m):
                            ps = mmp.tile([128, CH], F32, tag="mm", name="mm")
                            for ko in range(KO):
                                nc.tensor.matmul(
                                    ps,
                                    lhsT=wsb[:, ko, m * 128:(m + 1) * 128],
                                    rhs=xT_sb[c][:, ko, :],
                                    start=(ko == 0),
                                    stop=(ko == KO - 1),
                                )
                            nc.vector.tensor_copy(dst[m][c][:], ps)
                        groups.append(g)
                for tt in range(4):             # v tiles of this chunk
                    def g(tt=tt):
                        t = 4 * c + tt
                        ps = mmp.tile([128, CH], F32, tag="mm", name="mm")
                        for ko in range(KO):
                            nc.tensor.matmul(
                                ps[:, :HL * DH],
                                lhsT=xT_sb[c][:, ko, tt * 128:(tt + 1) * 128],
                                rhs=wv_sb[:, ko, :],
                                start=(ko == 0),
                                stop=(ko == KO - 1),
                            )
                        dst = v_sb[t][:].rearrange("p (h e) -> p h e", e=VW)
                        src = ps[:, :HL * DH].rearrange("p (h e) -> p h e", e=DH)
                        nc.vector.tensor_copy(dst[:, :, :DH], src)
                    groups.append(g)
                return groups

            def attention(c, pr, inject=()):
                """Causal attention for q-chunk c, head pair pr (2pr, 2pr+1).
                Emitters from `inject` are spread between t-steps so their PE
                work fills the gaps of this ACT-bound stretch. The pair's
                denominator rows are DMA'd dense onto partitions 0/1 of its
                own lb2 tile (so a pair can be finished independently)."""
                lb2 = stg.tile([2, CH], F32, tag=f"lb2{pr}", name=f"lb2{pr}")
                inject = list(inject)
                nk = 4 * c + 4                  # k tiles this chunk needs
                every = max(1, (nk + len(inject) - 1) // max(1, len(inject))) \
                    if inject else 0
                po = [ps2o.tile([65, CH], F32, tag=f"po{hh}", name=f"po{hh}")
                      for hh in range(2)]

                def emit_score(t):
                    """score pair matmuls + exp + causal triangle mask."""
                    d = t - 4 * c               # >= 0 on diagonal tiles
                    lo = 128 * max(d, 0)        # first valid column in chunk
                    st = ps2s.tile([128, 2, CH], F32, tag="s", name="s")
                    for hh in range(2):
                        nc.tensor.matmul(
                            st[:, hh, lo:],
                            lhsT=kT_sb[pr][t // 4][
                                hh * 64:hh * 64 + 64,
                                (t % 4) * 128:(t % 4) * 128 + 128],
                            rhs=qT_sb[pr][c][hh * 64:hh * 64 + 64, lo:],
                            start=True,
                            stop=True,
                        )
                    pt = ptp.tile([128, 2, CH], BF16, tag="pt", name="pt")
                    nc.scalar.activation(
                        pt[:, :, lo:], st[:, :, lo:], EXP,
                        bias=bias0[:, 0:1],
                        scale=float(DH) ** -0.5,
                    )
                    if d >= 0:                  # zero the k>q triangle, which
                        # only spans the first 128 columns of the valid range
                        nc.gpsimd.affine_select(
                            out=pt[:, :, lo:lo + 128],
                            in_=pt[:, :, lo:lo + 128],
                            compare_op=mybir.AluOpType.is_ge,
                            fill=0.0,
                            base=0,
                            pattern=[[0, 2], [1, 128]],
                            channel_multiplier=-1,
                        )
                    return pt, lo

                # software pipeline: scores run one t ahead of the PV matmuls
                # so the PE never sits behind the exp of the tile it consumes
                pts = {0: emit_score(0)}
                for t in range(nk):
                    if t + 1 < nk:
                        pts[t + 1] = emit_score(t + 1)
                    # injected PE work lands between the look-ahead score and
                    # this step's PV matmuls, covering the exp/mask wait
                    if inject and t % every == every - 1:
                        inject.pop(0)()
                    pt, lo = pts.pop(t)
                    for hh in range(2):
                        h = 2 * pr + hh
                        nc.tensor.matmul(
                            po[hh][:, lo:],
                            lhsT=v_sb[t][:, h * VW:(h + 1) * VW],
                            rhs=pt[:, hh, lo:],
                            start=(t == 0),
                            stop=(t == nk - 1),
                        )
                for g in inject:
                    g()
                # chain head: stage po to SBUF, move l rows dense onto
                # partitions 2pr/2pr+1 of the chunk's lb4 (SBUF->SBUF DMA
                # shifts partitions; compute engines cannot)
                ots = []
                for hh in range(2):
                    ot = stg.tile([65, CH], F32, tag=f"ot{pr}{hh}",
                                  name=f"ot{pr}{hh}")
                    nc.vector.tensor_copy(ot[64:65], po[hh][64:65])
                    nc.sync.dma_start(lb2[hh:hh + 1], ot[64:65])
                    nc.vector.tensor_copy(ot[0:64], po[hh][0:64])
                    ots.append(ot)
                return ots, lb2

            def fin_all(c, r0, r1):
                """batched finish of both pairs: one reciprocal over all 4
                denominator rows (gathered by two cheap SBUF->SBUF DMAs)."""
                lb4 = stg.tile([4, CH], F32, tag="lb4", name="lb4")
                nc.sync.dma_start(lb4[0:2], r0[1][:])
                nc.sync.dma_start(lb4[2:4], r1[1][:])
                rb = stg.tile([4, CH], F32, tag="rb4", name="rb4")
                nc.vector.reciprocal(rb, lb4)
                sr4 = dstg.tile([4, CH], F32, tag="sr4", name="sr4")
                nc.sync.dma_start(sr4[:], rb[:])
                for pr, rr in ((0, r0), (1, r1)):
                    for hh in (1, 0):
                        idx = 2 * pr + hh
                        rep = stg.tile([64, CH], F32, tag=f"rep{pr}{hh}",
                                       name=f"rep{pr}{hh}")
                        nc.sync.dma_start(
                            rep[:], sr4[idx:idx + 1].to_broadcast((64, CH)))
                        if hh == 0:
                            nc.vector.tensor_mul(
                                pj_sb[c][pr][0:64, :], rr[0][hh][0:64], rep[:])
                        else:
                            tmp = stg.tile([64, CH], BF16, tag=f"tmp{pr}",
                                           name=f"tmp{pr}")
                            nc.vector.tensor_mul(tmp, rr[0][hh][0:64], rep[:])
                            nc.sync.dma_start(pj_sb[c][pr][64:128, :], tmp)

            def fin_pair(c, pr, ots, lb2):
                """reciprocal of one pair's denominators + scale into pj.
                Returns the chain tiles (used as warm-up pegs at the tail)."""
                rb = stg.tile([2, CH], F32, tag=f"rb{pr}", name=f"rb{pr}")
                nc.vector.reciprocal(rb, lb2[:])
                sr2 = dstg.tile([2, CH], F32, tag=f"sr{pr}", name=f"sr{pr}")
                nc.sync.dma_start(sr2[:], rb[:])
                reps = []
                # odd head (needs the partition-shift DMA) first, so the
                # final dependency of pj is the cheap direct multiply
                for hh in (1, 0):
                    rep = stg.tile([64, CH], F32, tag=f"rep{pr}{hh}",
                                   name=f"rep{pr}{hh}")
                    nc.sync.dma_start(
                        rep[:], sr2[hh:hh + 1].to_broadcast((64, CH)))
                    reps.append(rep)
                    if hh == 0:
                        nc.vector.tensor_mul(
                            pj_sb[c][pr][0:64, :], ots[hh][0:64], rep[:])
                    else:
                        tmp = stg.tile([64, CH], BF16, tag=f"tmp{pr}",
                                       name=f"tmp{pr}")
                        nc.vector.tensor_mul(tmp, ots[hh][0:64], rep[:])
                        nc.sync.dma_start(pj_sb[c][pr][64:128, :], tmp)
                return rb, reps

            def p3_groups(c, ko_list=(0, 1), accum=False, tail=False):
                """partial output projection psum-groups for chunk c over the
                given pj pairs; accum=True adds into DRAM instead of writing
                (used to split the last chunk's projection per pair)."""
                groups = []
                for tt in range(4):
                    for n2 in range(2):
                        def g(tt=tt, n2=n2):
                            mt = 4 * c + tt
                            prt = mmp.tile([128, CH], F32, tag="mm", name="mm")
                            for j, ko in enumerate(ko_list):
                                nc.tensor.matmul(
                                    prt,
                                    lhsT=pj_sb[c][ko][:, tt * 128:tt * 128 + 128],
                                    rhs=wo_sb[:, ko, n2 * CH:(n2 + 1) * CH],
                                    start=(j == 0),
                                    stop=(j == len(ko_list) - 1),
                                )
                            ob = ostg.tile([128, CH], F32, tag="ob", name="ob")
                            if tail and n2 == 0:
                                nc.scalar.copy(ob, prt)
                            else:
                                nc.vector.tensor_copy(ob, prt)
                            nc.scalar.dma_start(
                                out_v[:, mt, n2 * CH:(n2 + 1) * CH], ob,
                                accum_op=(mybir.AluOpType.add if accum
                                          else mybir.AluOpType.bypass))
                        groups.append(g)
                return groups

            pending = {}
            tks = {}
            lc = NCH - 1

            def prepass_groups():
                """ko=0 (pair 0) half of the last chunk's projection, stashed
                in SBUF tk tiles; runs inside the last attention block."""
                groups = []
                for tt in range(4):
                    for n2 in range(2):
                        def g(tt=tt, n2=n2):
                            prt = mmp.tile([128, CH], F32, tag="mm", name="mm")
                            nc.tensor.matmul(
                                prt,
                                lhsT=pj_sb[lc][0][:, tt * 128:tt * 128 + 128],
                                rhs=wo_sb[:, 0, n2 * CH:(n2 + 1) * CH],
                                start=True, stop=True,
                            )
                            tk = tkp.tile([128, CH], F32, tag=f"tk{tt}{n2}",
                                          name=f"tk{tt}{n2}")
                            nc.vector.tensor_copy(tk, prt)
                            tks[(tt, n2)] = tk
                        groups.append(g)
                return groups

            for g in p1_groups(0):
                g()
            for c in range(NCH):
                inj0, inj1 = [], []
                if c > 0:
                    inj0.append(lambda c=c: fin_all(
                        c - 1, pending[(c - 1, 0)], pending[(c - 1, 1)]))
                if c + 1 < NCH:
                    p1 = p1_groups(c + 1)
                    inj0 += p1[:4]
                    inj1 += p1[4:]
                if c > 0:
                    inj1 += p3_groups(c - 1)
                r0 = attention(c, 0, inj0)
                if c == lc:
                    inj1.insert(0, lambda: fin_pair(lc, 0, *r0))
                    inj1 += prepass_groups()
                r1 = attention(c, 1, inj1)
                pending[(c, 0)] = r0
                pending[(c, 1)] = r1

            # ---- tail: finish pair 1, keep the PE warm through the chain
            # with dummy matmuls pegged to chain outputs, then the ko=1 half
            # of the projection + add of the stashed ko=0 half
            ots1, lb2_1 = pending[(lc, 1)]
            rb, reps = fin_pair(lc, 1, *pending[(lc, 1)])
            sdummy = ps2s.tile([128, 2, CH], F32, tag="s", name="s")
            for peg in (ots1[0], ots1[1], lb2_1, rb, reps[0], reps[1]):
                pk = min(peg.shape[0], 64)
                nc.tensor.matmul(
                    sdummy[:, 0, :],
                    lhsT=peg[0:pk, 0:128], rhs=peg[0:pk, :],
                    start=True, stop=True,
                )
            for tt in range(4):
                for n2 in range(2):
                    prt = mmp.tile([128, CH], F32, tag="mm", name="mm")
                    nc.tensor.matmul(
                        prt,
                        lhsT=pj_sb[lc][1][:, tt * 128:tt * 128 + 128],
                        rhs=wo_sb[:, 1, n2 * CH:(n2 + 1) * CH],
                        start=True, stop=True,
                    )
                    ob = ostg.tile([128, CH], F32, tag="ob", name="ob")
                    nc.vector.tensor_add(ob, prt, tks[(tt, n2)][:])
                    eng = nc.scalar if (2 * tt + n2) % 2 == 0 else nc.sync
                    eng.dma_start(
                        out_v[:, 4 * lc + tt, n2 * CH:(n2 + 1) * CH], ob)

    nc.compile()
    _cached["nc"] = nc
    return nc


def make_in_maps(x, w_qkv, w_out):
    bf = ml_dtypes.bfloat16
    in_maps = []
    for core in range(N_CORES):
        b, h0 = core // 4, (core % 4) * HL
        c0 = h0 * DH
        in_maps.append({
            "xT": np.ascontiguousarray(x[b].T).astype(bf),
            "wq": w_qkv[:, c0:c0 + HL * DH].astype(bf),
            "wk": w_qkv[:, D + c0:D + c0 + HL * DH].astype(bf),
            "wv": w_qkv[:, 2 * D + c0:2 * D + c0 + HL * DH].astype(bf),
            "wo": w_out[c0:c0 + HL * DH, :].astype(bf),
        })
    return in_maps


def run_sharded(x, w_qkv, w_out, trace=False):
    nc = build()
    res = run_bass_kernel_spmd(
        nc, make_in_maps(x, w_qkv, w_out), core_ids=list(range(N_CORES)),
        trace=trace,
    )
    out = np.zeros((B, S, D), np.float32)
    for core in range(N_CORES):
        out[core // 4] += res.results[core]["out"]
    return out, res.exec_time_ns


def kernel(x, w_qkv, w_out):
    out, _ = run_sharded(x, w_qkv, w_out)
    return out


# revision 38
# speedup vs baseline: 1.0176x; 1.0147x over previous
"""Causal multi-head attention (B=2, S=2048, D=1024, H=16) on 8 trn2 NeuronCores.

Sharding: data-parallel over batch (2 groups of 4 cores), tensor-parallel over
heads within a group (4 heads/core). Each core computes qkv projection for its
head slice, causal flash-style attention, and a partial output projection;
the host sums the 4 partials per batch element.

Device-side layout notes (per core):
  - x arrives pre-transposed (host does x[b].T, cast bf16): xT [D=1024, S=2048].
  - q,k are produced transposed (qT/kT [head_dim, S]) so score matmuls
    contract head_dim on partitions: scores^T [Sk, Sq] = kT_tile^T @ qT.
    Head pairs sit at partition bases 0/64, so their K=64 score matmuls run
    concurrently in distinct PE row groups.
  - softmax denominators come from a ones-column appended to V (M=65 in the
    PV matmul); 1/l is one batched DVE reciprocal per head pair, broadcast
    across partitions by a DRAM-bounce DMA. Nothing lands on the PE queue.
  - causal masking zeroes the k>q triangle of exp(scores) on GPSIMD; exp
    needs no max-subtraction (|score| < ~4). Diagonal tiles restrict all
    work (matmul N, exp, mask) to the valid column range.
  - everything is emitted interleaved per q-chunk (projection c, attention c,
    output-projection c-1) so ACT exp work overlaps PE projection work.
"""

import numpy as np
import ml_dtypes

import concourse.bass as bass
import concourse.mybir as mybir
import concourse.tile as tile
from concourse import bacc
from concourse.bass_utils import run_bass_kernel_spmd

B, S, D, H = 2, 2048, 1024, 16
DH = D // H              # 64
HL = 4                   # heads per core
N_CORES = 8
KO = D // 128            # 8 contraction subtiles for the qkv projection
CH = 512                 # q chunk (matmul moving dim)
NCH = S // CH            # 4
KT = S // 128            # 16 k tiles
VW = DH + 1              # v columns incl the ones column
F32 = mybir.dt.float32
BF16 = mybir.dt.bfloat16
EXP = mybir.ActivationFunctionType.Exp

_cached = {}


def build():
    if "nc" in _cached:
        return _cached["nc"]

    nc = bacc.Bacc("TRN2", target_bir_lowering=False, debug=False)

    xT = nc.dram_tensor("xT", [D, S], BF16, kind="ExternalInput")
    wq = nc.dram_tensor("wq", [D, HL * DH], BF16, kind="ExternalInput")
    wk = nc.dram_tensor("wk", [D, HL * DH], BF16, kind="ExternalInput")
    wv = nc.dram_tensor("wv", [D, HL * DH], BF16, kind="ExternalInput")
    wo = nc.dram_tensor("wo", [HL * DH, D], BF16, kind="ExternalInput")
    out = nc.dram_tensor("out", [S, D], F32, kind="ExternalOutput")

    xT_v = xT[:].rearrange("(ko p) s -> p ko s", p=128)      # [128, 8, 2048]
    wq_v = wq[:].rearrange("(ko p) m -> p ko m", p=128)      # [128, 8, 256]
    wk_v = wk[:].rearrange("(ko p) m -> p ko m", p=128)
    wv_v = wv[:].rearrange("(ko p) m -> p ko m", p=128)
    wo_v = wo[:].rearrange("(ko p) n -> p ko n", p=128)      # [128, 2, 1024]
    out_v = out[:].rearrange("(t p) n -> p t n", p=128)      # [128, 16, 1024]

    with tile.TileContext(nc) as tc:
        with (
            tc.tile_pool(name="persist", bufs=1) as pp,
            tc.tile_pool(name="mm", bufs=2, space=bass.MemorySpace.PSUM) as mmp,
            tc.tile_pool(name="ps2s", bufs=2, space=bass.MemorySpace.PSUM) as ps2s,
            tc.tile_pool(name="ps2o", bufs=1, space=bass.MemorySpace.PSUM) as ps2o,
            tc.tile_pool(name="ptp", bufs=3) as ptp,
            tc.tile_pool(name="stg", bufs=3) as stg,
            tc.tile_pool(name="ostg", bufs=4) as ostg,
            tc.tile_pool(name="dstg", bufs=4, space="DRAM") as dstg,
            tc.tile_pool(name="tkp", bufs=1) as tkp,
        ):
            # per-chunk tiles so later phases start as soon as inputs land
            xT_sb = [pp.tile([128, KO, CH], BF16, tag=f"xT{c}", name=f"xT{c}")
                     for c in range(NCH)]
            wq_sb = pp.tile([128, KO, HL * DH], BF16, tag="wq")
            wk_sb = pp.tile([128, KO, HL * DH], BF16, tag="wk")
            wv_sb = pp.tile([128, KO, HL * DH], BF16, tag="wv")
            wo_sb = pp.tile([128, 2, D], BF16, tag="wo")
            qT_sb = [[pp.tile([128, CH], BF16, tag=f"qT{m}{c}", name=f"qT{m}{c}")
                      for c in range(NCH)] for m in range(2)]
            kT_sb = [[pp.tile([128, CH], BF16, tag=f"kT{m}{c}", name=f"kT{m}{c}")
                      for c in range(NCH)] for m in range(2)]
            # v with a ones column appended per head: [v_h (64) | 1]
            v_sb = [pp.tile([128, HL * VW], BF16, tag=f"v{t}", name=f"v{t}")
                    for t in range(KT)]
            # normalized attention output, transposed: proj lhsT, per q-chunk
            pj_sb = [[pp.tile([128, CH], BF16, tag=f"pj{c}{p}", name=f"pj{c}{p}")
                      for p in range(2)] for c in range(NCH)]

            # first matmul needs wk + xT[0]: load those first
            nc.scalar.dma_start(wk_sb[:], wk_v)
            nc.sync.dma_start(xT_sb[0][:, 0:4, :], xT_v[:, 0:4, 0:CH])
            nc.scalar.dma_start(xT_sb[0][:, 4:8, :], xT_v[:, 4:8, 0:CH])
            nc.scalar.dma_start(wq_sb[:], wq_v)
            nc.scalar.dma_start(wv_sb[:], wv_v)
            nc.scalar.dma_start(wo_sb[:], wo_v)
            for c in range(1, NCH):
                nc.sync.dma_start(xT_sb[c][:], xT_v[:, :, c * CH:(c + 1) * CH])
            bias0 = pp.tile([128, 1], F32, tag="bias0")
            nc.gpsimd.memset(bias0[:], 0.0)
            for t in range(KT):
                nc.gpsimd.memset(v_sb[t][:], 1.0)

            def p1_groups(c):
                """qT, kT, v projection psum-groups for chunk c (injectable)."""
                groups = []
                for wsb, dst in ((wk_sb, kT_sb), (wq_sb, qT_sb)):
                    for m in range(2):          # 128-col tiles (2 heads each)
                        def g(wsb=wsb, dst=dst, m=m):
                            ps = mmp.tile([128, CH], F32, tag="mm", name="mm")
                            for ko in range(KO):
                                nc.tensor.matmul(
                                    ps,
                                    lhsT=wsb[:, ko, m * 128:(m + 1) * 128],
                                    rhs=xT_sb[c][:, ko, :],
                                    start=(ko == 0),
                                    stop=(ko == KO - 1),
                                )
                            nc.vector.tensor_copy(dst[m][c][:], ps)
                        groups.append(g)
                for tt in range(4):             # v tiles of this chunk
                    def g(tt=tt):
                        t = 4 * c + tt
                        ps = mmp.tile([128, CH], F32, tag="mm", name="mm")
                        for ko in range(KO):
                            nc.tensor.matmul(
                                ps[:, :HL * DH],
                                lhsT=xT_sb[c][:, ko, tt * 128:(tt + 1) * 128],
                                rhs=wv_sb[:, ko, :],
                                start=(ko == 0),
                                stop=(ko == KO - 1),
                            )
                        dst = v_sb[t][:].rearrange("p (h e) -> p h e", e=VW)
                        src = ps[:, :HL * DH].rearrange("p (h e) -> p h e", e=DH)
                        nc.vector.tensor_copy(dst[:, :, :DH], src)
                    groups.append(g)
                return groups

            def attention(c, pr, inject=()):
                """Causal attention for q-chunk c, head pair pr (2pr, 2pr+1).
                Emitters from `inject` are spread between t-steps so their PE
                work fills the gaps of this ACT-bound stretch. The pair's
                denominator rows are DMA'd dense onto partitions 0/1 of its
                own lb2 tile (so a pair can be finished independently)."""
                lb2 = stg.tile([2, CH], F32, tag=f"lb2{pr}", name=f"lb2{pr}")
                inject = list(inject)
                nk = 4 * c + 4                  # k tiles this chunk needs
                every = max(1, (nk + len(inject) - 1) // max(1, len(inject))) \
                    if inject else 0
                po = [ps2o.tile([65, CH], F32, tag=f"po{hh}", name=f"po{hh}")
                      for hh in range(2)]

                def emit_score(t):
                    """score pair matmuls + exp + causal triangle mask."""
                    d = t - 4 * c               # >= 0 on diagonal tiles
                    lo = 128 * max(d, 0)        # first valid column in chunk
                    st = ps2s.tile([128, 2, CH], F32, tag="s", name="s")
                    for hh in range(2):
                        nc.tensor.matmul(
                            st[:, hh, lo:],
                            lhsT=kT_sb[pr][t // 4][
                                hh * 64:hh * 64 + 64,
                                (t % 4) * 128:(t % 4) * 128 + 128],
                            rhs=qT_sb[pr][c][hh * 64:hh * 64 + 64, lo:],
                            start=True,
                            stop=True,
                        )
                    pt = ptp.tile([128, 2, CH], BF16, tag="pt", name="pt")
                    nc.scalar.activation(
                        pt[:, :, lo:], st[:, :, lo:], EXP,
                        bias=bias0[:, 0:1],
                        scale=float(DH) ** -0.5,
                    )
                    if d >= 0:                  # zero the k>q triangle, which
                        # only spans the first 128 columns of the valid range
                        nc.gpsimd.affine_select(
                            out=pt[:, :, lo:lo + 128],
                            in_=pt[:, :, lo:lo + 128],
                            compare_op=mybir.AluOpType.is_ge,
                            fill=0.0,
                            base=0,
                            pattern=[[0, 2], [1, 128]],
                            channel_multiplier=-1,
                        )
                    return pt, lo

                # software pipeline: scores run one t ahead of the PV matmuls
                # so the PE never sits behind the exp of the tile it consumes
                pts = {0: emit_score(0)}
                for t in range(nk):
                    if t + 1 < nk:
                        pts[t + 1] = emit_score(t + 1)
                    # injected PE work lands between the look-ahead score and
                    # this step's PV matmuls, covering the exp/mask wait
                    if inject and t % every == every - 1:
                        inject.pop(0)()
                    pt, lo = pts.pop(t)
                    for hh in range(2):
                        h = 2 * pr + hh
                        nc.tensor.matmul(
                            po[hh][:, lo:],
                            lhsT=v_sb[t][:, h * VW:(h + 1) * VW],
                            rhs=pt[:, hh, lo:],
                            start=(t == 0),
                            stop=(t == nk - 1),
                        )
                for g in inject:
                    g()
                # chain head: stage po to SBUF, move l rows dense onto
                # partitions 2pr/2pr+1 of the chunk's lb4 (SBUF->SBUF DMA
                # shifts partitions; compute engines cannot)
                ots = []
                for hh in range(2):
                    ot = stg.tile([65, CH], F32, tag=f"ot{pr}{hh}",
                                  name=f"ot{pr}{hh}")
                    nc.vector.tensor_copy(ot[64:65], po[hh][64:65])
                    nc.sync.dma_start(lb2[hh:hh + 1], ot[64:65])
                    nc.vector.tensor_copy(ot[0:64], po[hh][0:64])
                    ots.append(ot)
                return ots, lb2

            def fin_all(c, r0, r1):
                """batched finish of both pairs: one reciprocal over all 4
                denominator rows (gathered by two cheap SBUF->SBUF DMAs)."""
                lb4 = stg.tile([4, CH], F32, tag="lb4", name="lb4")
                nc.sync.dma_start(lb4[0:2], r0[1][:])
                nc.sync.dma_start(lb4[2:4], r1[1][:])
                rb = stg.tile([4, CH], F32, tag="rb4", name="rb4")
                nc.vector.reciprocal(rb, lb4)
                sr4 = dstg.tile([4, CH], F32, tag="sr4", name="sr4")
                nc.sync.dma_start(sr4[:], rb[:])
                for pr, rr in ((0, r0), (1, r1)):
                    for hh in (1, 0):
                        idx = 2 * pr + hh
                        rep = stg.tile([64, CH], F32, tag=f"rep{pr}{hh}",
                                       name=f"rep{pr}{hh}")
                        nc.sync.dma_start(
                            rep[:], sr4[idx:idx + 1].to_broadcast((64, CH)))
                        if hh == 0:
                            nc.vector.tensor_mul(
                                pj_sb[c][pr][0:64, :], rr[0][hh][0:64], rep[:])
                        else:
                            tmp = stg.tile([64, CH], BF16, tag=f"tmp{pr}",
                                           name=f"tmp{pr}")
                            nc.vector.tensor_mul(tmp, rr[0][hh][0:64], rep[:])
                            nc.sync.dma_start(pj_sb[c][pr][64:128, :], tmp)

            def fin_pair(c, pr, ots, lb2):
                """reciprocal of one pair's denominators + scale into pj.
                Returns the chain tiles (used as warm-up pegs at the tail)."""
                rb = stg.tile([2, CH], F32, tag=f"rb{pr}", name=f"rb{pr}")
                nc.vector.reciprocal(rb, lb2[:])
                sr2 = dstg.tile([2, CH], F32, tag=f"sr{pr}", name=f"sr{pr}")
                nc.sync.dma_start(sr2[:], rb[:])
                reps = []
                # odd head (needs the partition-shift DMA) first, so the
                # final dependency of pj is the cheap direct multiply
                for hh in (1, 0):
                    rep = stg.tile([64, CH], F32, tag=f"rep{pr}{hh}",
                                   name=f"rep{pr}{hh}")
                    nc.sync.dma_start(
                        rep[:], sr2[hh:hh + 1].to_broadcast((64, CH)))
                    reps.append(rep)
                    if hh == 0:
                        nc.vector.tensor_mul(
                            pj_sb[c][pr][0:64, :], ots[hh][0:64], rep[:])
                    else:
                        tmp = stg.tile([64, CH], BF16, tag=f"tmp{pr}",
                                       name=f"tmp{pr}")
                        nc.vector.tensor_mul(tmp, ots[hh][0:64], rep[:])
                        nc.sync.dma_start(pj_sb[c][pr][64:128, :], tmp)
                return rb, reps

            def p3_groups(c, ko_list=(0, 1), accum=False, tail=False):
                """partial output projection psum-groups for chunk c over the
                given pj pairs; accum=True adds into DRAM instead of writing
                (used to split the last chunk's projection per pair)."""
                groups = []
                for tt in range(4):
                    for n2 in range(2):
                        def g(tt=tt, n2=n2):
                            mt = 4 * c + tt
                            prt = mmp.tile([128, CH], F32, tag="mm", name="mm")
                            for j, ko in enumerate(ko_list):
                                nc.tensor.matmul(
                                    prt,
                                    lhsT=pj_sb[c][ko][:, tt * 128:tt * 128 + 128],
                                    rhs=wo_sb[:, ko, n2 * CH:(n2 + 1) * CH],
                                    start=(j == 0),
                                    stop=(j == len(ko_list) - 1),
                                )
                            ob = ostg.tile([128, CH], F32, tag="ob", name="ob")
                            if tail and n2 == 0:
                                nc.scalar.copy(ob, prt)
                            else:
                                nc.vector.tensor_copy(ob, prt)
                            nc.scalar.dma_start(
                                out_v[:, mt, n2 * CH:(n2 + 1) * CH], ob,
                                accum_op=(mybir.AluOpType.add if accum
                                          else mybir.AluOpType.bypass))
                        groups.append(g)
                return groups

            pending = {}
            tks = {}
            lc = NCH - 1

            def prepass_groups():
                """ko=0 (pair 0) half of the last chunk's projection, stashed
                in SBUF tk tiles; runs inside the last attention block."""
                groups = []
                for tt in range(4):
                    for n2 in range(2):
                        def g(tt=tt, n2=n2):
                            prt = mmp.tile([128, CH], F32, tag="mm", name="mm")
                            nc.tensor.matmul(
                                prt,
                                lhsT=pj_sb[lc][0][:, tt * 128:tt * 128 + 128],
                                rhs=wo_sb[:, 0, n2 * CH:(n2 + 1) * CH],
                                start=True, stop=True,
                            )
                            tk = tkp.tile([128, CH], F32, tag=f"tk{tt}{n2}",
                                          name=f"tk{tt}{n2}")
                            nc.vector.tensor_copy(tk, prt)
                            tks[(tt, n2)] = tk
                        groups.append(g)
                return groups

            for g in p1_groups(0):
                g()
            for c in range(NCH):
                inj0, inj1 = [], []
                if c > 0:
                    inj0.append(lambda c=c: fin_all(
                        c - 1, pending[(c - 1, 0)], pending[(c - 1, 1)]))
                if c + 1 < NCH:
                    p1 = p1_groups(c + 1)
                    inj0 += p1[:4]
                    inj1 += p1[4:]
                if c > 0:
                    inj1 += p3_groups(c - 1)
                r0 = attention(c, 0, inj0)
                if c == lc:
                    inj1.insert(0, lambda: fin_pair(lc, 0, *r0))
                    inj1 += prepass_groups()
                r1 = attention(c, 1, inj1)
                pending[(c, 0)] = r0
                pending[(c, 1)] = r1

            # ---- tail: finish pair 1, keep the PE warm through the chain
            # with dummy matmuls pegged to chain outputs, then the ko=1 half
            # of the projection + add of the stashed ko=0 half
            ots1, lb2_1 = pending[(lc, 1)]
            rb, reps = fin_pair(lc, 1, *pending[(lc, 1)])
            sdummy = ps2s.tile([128, 2, CH], F32, tag="s", name="s")
            for peg in (ots1[0], ots1[1], lb2_1, rb, reps[0], reps[1]):
                pk = min(peg.shape[0], 64)
                nc.tensor.matmul(
                    sdummy[:, 0, :],
                    lhsT=peg[0:pk, 0:128], rhs=peg[0:pk, :],
                    start=True, stop=True,
                )
            for tt in range(4):
                for n2 in range(2):
                    prt = mmp.tile([128, CH], F32, tag="mm", name="mm")
                    nc.tensor.matmul(
                        prt,
                        lhsT=pj_sb[lc][1][:, tt * 128:tt * 128 + 128],
                        rhs=wo_sb[:, 1, n2 * CH:(n2 + 1) * CH],
                        start=True, stop=True,
                    )
                    ob = ostg.tile([128, CH], F32, tag="ob", name="ob")
                    nc.vector.tensor_add(ob, prt, tks[(tt, n2)][:])
                    eng = nc.scalar if (2 * tt + n2) % 2 == 0 else nc.sync
                    eng.dma_start(
                        out_v[:, 4 * lc + tt, n2 * CH:(n2 + 1) * CH], ob)

    nc.compile()
    _cached["nc"] = nc
    return nc


def make_in_maps(x, w_qkv, w_out):
    bf = ml_dtypes.bfloat16
    in_maps = []
    for core in range(N_CORES):
        b, h0 = core // 4, (core % 4) * HL
        c0 = h0 * DH
        in_maps.append({
            "xT": np.ascontiguousarray(x[b].T).astype(bf),
            "wq": w_qkv[:, c0:c0 + HL * DH].astype(bf),
            "wk": w_qkv[:, D + c0:D + c0 + HL * DH].astype(bf),
            "wv": w_qkv[:, 2 * D + c0:2 * D + c0 + HL * DH].astype(bf),
            "wo": w_out[c0:c0 + HL * DH, :].astype(bf),
        })
    return in_maps


def run_sharded(x, w_qkv, w_out, trace=False):
    nc = build()
    res = run_bass_kernel_spmd(
        nc, make_in_maps(x, w_qkv, w_out), core_ids=list(range(N_CORES)),
        trace=trace,
    )
    out = np.zeros((B, S, D), np.float32)
    for core in range(N_CORES):
        out[core // 4] += res.results[core]["out"]
    return out, res.exec_time_ns


def kernel(x, w_qkv, w_out):
    out, _ = run_sharded(x, w_qkv, w_out)
    return out


# revision 41
# speedup vs baseline: 1.0382x; 1.0202x over previous
"""Causal multi-head attention (B=2, S=2048, D=1024, H=16) on 8 trn2 NeuronCores.

Sharding: data-parallel over batch (2 groups of 4 cores), tensor-parallel over
heads within a group (4 heads/core). Each core computes qkv projection for its
head slice, causal flash-style attention, and a partial output projection;
the host sums the 4 partials per batch element.

Device-side layout notes (per core):
  - x arrives pre-transposed (host does x[b].T, cast bf16): xT [D=1024, S=2048].
  - q,k are produced transposed (qT/kT [head_dim, S]) so score matmuls
    contract head_dim on partitions: scores^T [Sk, Sq] = kT_tile^T @ qT.
    Head pairs sit at partition bases 0/64, so their K=64 score matmuls run
    concurrently in distinct PE row groups.
  - softmax denominators come from a ones-column appended to V (M=65 in the
    PV matmul); 1/l is one batched DVE reciprocal per head pair, broadcast
    across partitions by a DRAM-bounce DMA. Nothing lands on the PE queue.
  - causal masking zeroes the k>q triangle of exp(scores) on GPSIMD; exp
    needs no max-subtraction (|score| < ~4). Diagonal tiles restrict all
    work (matmul N, exp, mask) to the valid column range.
  - everything is emitted interleaved per q-chunk (projection c, attention c,
    output-projection c-1) so ACT exp work overlaps PE projection work.
"""

import numpy as np
import ml_dtypes

import concourse.bass as bass
import concourse.mybir as mybir
import concourse.tile as tile
from concourse import bacc
from concourse.bass_utils import run_bass_kernel_spmd
from concourse.masks import make_identity

B, S, D, H = 2, 2048, 1024, 16
DH = D // H              # 64
HL = 4                   # heads per core
N_CORES = 8
KO = D // 128            # 8 contraction subtiles for the qkv projection
CH = 512                 # q chunk (matmul moving dim)
NCH = S // CH            # 4
KT = S // 128            # 16 k tiles
VW = DH + 1              # v columns incl the ones column
F32 = mybir.dt.float32
BF16 = mybir.dt.bfloat16
EXP = mybir.ActivationFunctionType.Exp

_cached = {}


def build():
    if "nc" in _cached:
        return _cached["nc"]

    nc = bacc.Bacc("TRN2", target_bir_lowering=False, debug=False)

    xT = nc.dram_tensor("xT", [D, S], BF16, kind="ExternalInput")
    wq = nc.dram_tensor("wq", [D, HL * DH], BF16, kind="ExternalInput")
    wk = nc.dram_tensor("wk", [D, HL * DH], BF16, kind="ExternalInput")
    wv = nc.dram_tensor("wv", [D, HL * DH], BF16, kind="ExternalInput")
    wo = nc.dram_tensor("wo", [HL * DH, D], BF16, kind="ExternalInput")
    out = nc.dram_tensor("out", [S, D], F32, kind="ExternalOutput")

    xT_v = xT[:].rearrange("(ko p) s -> p ko s", p=128)      # [128, 8, 2048]
    wq_v = wq[:].rearrange("(ko p) m -> p ko m", p=128)      # [128, 8, 256]
    wk_v = wk[:].rearrange("(ko p) m -> p ko m", p=128)
    wv_v = wv[:].rearrange("(ko p) m -> p ko m", p=128)
    wo_v = wo[:].rearrange("(ko p) n -> p ko n", p=128)      # [128, 2, 1024]
    out_v = out[:].rearrange("(t p) n -> p t n", p=128)      # [128, 16, 1024]

    with tile.TileContext(nc) as tc:
        with (
            tc.tile_pool(name="persist", bufs=1) as pp,
            tc.tile_pool(name="mm", bufs=2, space=bass.MemorySpace.PSUM) as mmp,
            tc.tile_pool(name="ps2s", bufs=2, space=bass.MemorySpace.PSUM) as ps2s,
            tc.tile_pool(name="ps2o", bufs=1, space=bass.MemorySpace.PSUM) as ps2o,
            tc.tile_pool(name="ptp", bufs=3) as ptp,
            tc.tile_pool(name="stg", bufs=3) as stg,
            tc.tile_pool(name="ostg", bufs=4) as ostg,
            tc.tile_pool(name="dstg", bufs=4, space="DRAM") as dstg,
            tc.tile_pool(name="tkp", bufs=1) as tkp,
        ):
            # per-chunk tiles so later phases start as soon as inputs land
            xT_sb = [pp.tile([128, KO, CH], BF16, tag=f"xT{c}", name=f"xT{c}")
                     for c in range(NCH)]
            wq_sb = pp.tile([128, KO, HL * DH], BF16, tag="wq")
            wk_sb = pp.tile([128, KO, HL * DH], BF16, tag="wk")
            wv_sb = pp.tile([128, KO, HL * DH], BF16, tag="wv")
            wo_sb = pp.tile([128, 2, D], BF16, tag="wo")
            qT_sb = [[pp.tile([128, CH], BF16, tag=f"qT{m}{c}", name=f"qT{m}{c}")
                      for c in range(NCH)] for m in range(2)]
            kT_sb = [[pp.tile([128, CH], BF16, tag=f"kT{m}{c}", name=f"kT{m}{c}")
                      for c in range(NCH)] for m in range(2)]
            # v with a ones column appended per head: [v_h (64) | 1]
            v_sb = [pp.tile([128, HL * VW], BF16, tag=f"v{t}", name=f"v{t}")
                    for t in range(KT)]
            # normalized attention output, transposed: proj lhsT, per q-chunk
            pj_sb = [[pp.tile([128, CH], BF16, tag=f"pj{c}{p}", name=f"pj{c}{p}")
                      for p in range(2)] for c in range(NCH)]

            # first matmul needs wk + xT[0]: load those first
            nc.scalar.dma_start(wk_sb[:], wk_v)
            nc.sync.dma_start(xT_sb[0][:, 0:4, :], xT_v[:, 0:4, 0:CH])
            nc.scalar.dma_start(xT_sb[0][:, 4:8, :], xT_v[:, 4:8, 0:CH])
            nc.scalar.dma_start(wq_sb[:], wq_v)
            nc.scalar.dma_start(wv_sb[:], wv_v)
            nc.scalar.dma_start(wo_sb[:], wo_v)
            for c in range(1, NCH):
                nc.sync.dma_start(xT_sb[c][:], xT_v[:, :, c * CH:(c + 1) * CH])
            bias0 = pp.tile([128, 1], F32, tag="bias0")
            nc.gpsimd.memset(bias0[:], 0.0)
            idf = pp.tile([128, 128], F32, tag="idf")
            make_identity(nc, idf[:])
            for t in range(KT):
                nc.gpsimd.memset(v_sb[t][:], 1.0)

            def p1_groups(c):
                """qT, kT, v projection psum-groups for chunk c (injectable)."""
                groups = []
                for wsb, dst in ((wk_sb, kT_sb), (wq_sb, qT_sb)):
                    for m in range(2):          # 128-col tiles (2 heads each)
                        def g(wsb=wsb, dst=dst, m=m):
                            ps = mmp.tile([128, CH], F32, tag="mm", name="mm")
                            for ko in range(KO):
                                nc.tensor.matmul(
                                    ps,
                                    lhsT=wsb[:, ko, m * 128:(m + 1) * 128],
                                    rhs=xT_sb[c][:, ko, :],
                                    start=(ko == 0),
                                    stop=(ko == KO - 1),
                                )
                            nc.vector.tensor_copy(dst[m][c][:], ps)
                        groups.append(g)
                for tt in range(4):             # v tiles of this chunk
                    def g(tt=tt):
                        t = 4 * c + tt
                        ps = mmp.tile([128, CH], F32, tag="mm", name="mm")
                        for ko in range(KO):
                            nc.tensor.matmul(
                                ps[:, :HL * DH],
                                lhsT=xT_sb[c][:, ko, tt * 128:(tt + 1) * 128],
                                rhs=wv_sb[:, ko, :],
                                start=(ko == 0),
                                stop=(ko == KO - 1),
                            )
                        dst = v_sb[t][:].rearrange("p (h e) -> p h e", e=VW)
                        src = ps[:, :HL * DH].rearrange("p (h e) -> p h e", e=DH)
                        nc.vector.tensor_copy(dst[:, :, :DH], src)
                    groups.append(g)
                return groups

            def attention(c, pr, inject=(), otfull=False):
                """Causal attention for q-chunk c, head pair pr (2pr, 2pr+1).
                Emitters from `inject` are spread between t-steps so their PE
                work fills the gaps of this ACT-bound stretch. The pair's
                denominator rows are DMA'd dense onto partitions 0/1 of its
                own lb2 tile (so a pair can be finished independently)."""
                lb2 = stg.tile([2, CH], F32, tag=f"lb2{pr}", name=f"lb2{pr}")
                inject = list(inject)
                nk = 4 * c + 4                  # k tiles this chunk needs
                every = max(1, (nk + len(inject) - 1) // max(1, len(inject))) \
                    if inject else 0
                po = [ps2o.tile([65, CH], F32, tag=f"po{hh}", name=f"po{hh}")
                      for hh in range(2)]

                def emit_score(t):
                    """score pair matmuls + exp + causal triangle mask."""
                    d = t - 4 * c               # >= 0 on diagonal tiles
                    lo = 128 * max(d, 0)        # first valid column in chunk
                    st = ps2s.tile([128, 2, CH], F32, tag="s", name="s")
                    for hh in range(2):
                        nc.tensor.matmul(
                            st[:, hh, lo:],
                            lhsT=kT_sb[pr][t // 4][
                                hh * 64:hh * 64 + 64,
                                (t % 4) * 128:(t % 4) * 128 + 128],
                            rhs=qT_sb[pr][c][hh * 64:hh * 64 + 64, lo:],
                            start=True,
                            stop=True,
                        )
                    pt = ptp.tile([128, 2, CH], BF16, tag="pt", name="pt")
                    nc.scalar.activation(
                        pt[:, :, lo:], st[:, :, lo:], EXP,
                        bias=bias0[:, 0:1],
                        scale=float(DH) ** -0.5,
                    )
                    if d >= 0:                  # zero the k>q triangle, which
                        # only spans the first 128 columns of the valid range
                        nc.gpsimd.affine_select(
                            out=pt[:, :, lo:lo + 128],
                            in_=pt[:, :, lo:lo + 128],
                            compare_op=mybir.AluOpType.is_ge,
                            fill=0.0,
                            base=0,
                            pattern=[[0, 2], [1, 128]],
                            channel_multiplier=-1,
                        )
                    return pt, lo

                # software pipeline: scores run one t ahead of the PV matmuls
                # so the PE never sits behind the exp of the tile it consumes
                pts = {0: emit_score(0)}
                for t in range(nk):
                    if t + 1 < nk:
                        pts[t + 1] = emit_score(t + 1)
                    # injected PE work lands between the look-ahead score and
                    # this step's PV matmuls, covering the exp/mask wait
                    if inject and t % every == every - 1:
                        inject.pop(0)()
                    pt, lo = pts.pop(t)
                    for hh in range(2):
                        h = 2 * pr + hh
                        nc.tensor.matmul(
                            po[hh][:, lo:],
                            lhsT=v_sb[t][:, h * VW:(h + 1) * VW],
                            rhs=pt[:, hh, lo:],
                            start=(t == 0),
                            stop=(t == nk - 1),
                        )
                for g in inject:
                    g()
                # chain head: stage po to SBUF, move l rows dense onto
                # partitions 2pr/2pr+1 of the chunk's lb4 (SBUF->SBUF DMA
                # shifts partitions; compute engines cannot)
                ots = []
                for hh in range(2):
                    if otfull:
                        ot = tkp.tile([128, CH], F32, tag=f"otF{hh}",
                                      name=f"otF{hh}")
                    else:
                        ot = stg.tile([65, CH], F32, tag=f"ot{pr}{hh}",
                                      name=f"ot{pr}{hh}")
                    nc.vector.tensor_copy(ot[64:65], po[hh][64:65])
                    nc.sync.dma_start(lb2[hh:hh + 1], ot[64:65])
                    nc.vector.tensor_copy(ot[0:64], po[hh][0:64])
                    ots.append(ot)
                return ots, lb2

            def fin_all(c, r0, r1):
                """batched finish of both pairs: one reciprocal over all 4
                denominator rows (gathered by two cheap SBUF->SBUF DMAs)."""
                lb4 = stg.tile([4, CH], F32, tag="lb4", name="lb4")
                nc.sync.dma_start(lb4[0:2], r0[1][:])
                nc.sync.dma_start(lb4[2:4], r1[1][:])
                rb = stg.tile([4, CH], F32, tag="rb4", name="rb4")
                nc.vector.reciprocal(rb, lb4)
                sr4 = dstg.tile([4, CH], F32, tag="sr4", name="sr4")
                nc.sync.dma_start(sr4[:], rb[:])
                for pr, rr in ((0, r0), (1, r1)):
                    for hh in (1, 0):
                        idx = 2 * pr + hh
                        rep = stg.tile([64, CH], F32, tag=f"rep{pr}{hh}",
                                       name=f"rep{pr}{hh}")
                        nc.sync.dma_start(
                            rep[:], sr4[idx:idx + 1].to_broadcast((64, CH)))
                        if hh == 0:
                            nc.vector.tensor_mul(
                                pj_sb[c][pr][0:64, :], rr[0][hh][0:64], rep[:])
                        else:
                            tmp = stg.tile([64, CH], BF16, tag=f"tmp{pr}",
                                           name=f"tmp{pr}")
                            nc.vector.tensor_mul(tmp, rr[0][hh][0:64], rep[:])
                            nc.sync.dma_start(pj_sb[c][pr][64:128, :], tmp)

            def fin_pair(c, pr, ots, lb2):
                """reciprocal of one pair's denominators + scale into pj.
                Returns the chain tiles (used as warm-up pegs at the tail)."""
                rb = stg.tile([2, CH], F32, tag=f"rb{pr}", name=f"rb{pr}")
                nc.vector.reciprocal(rb, lb2[:])
                sr2 = dstg.tile([2, CH], F32, tag=f"sr{pr}", name=f"sr{pr}")
                nc.sync.dma_start(sr2[:], rb[:])
                reps = []
                # odd head (needs the partition-shift DMA) first, so the
                # final dependency of pj is the cheap direct multiply
                for hh in (1, 0):
                    rep = stg.tile([64, CH], F32, tag=f"rep{pr}{hh}",
                                   name=f"rep{pr}{hh}")
                    nc.sync.dma_start(
                        rep[:], sr2[hh:hh + 1].to_broadcast((64, CH)))
                    reps.append(rep)
                    if hh == 0:
                        nc.vector.tensor_mul(
                            pj_sb[c][pr][0:64, :], ots[hh][0:64], rep[:])
                    else:
                        tmp = stg.tile([64, CH], BF16, tag=f"tmp{pr}",
                                       name=f"tmp{pr}")
                        nc.vector.tensor_mul(tmp, ots[hh][0:64], rep[:])
                        nc.sync.dma_start(pj_sb[c][pr][64:128, :], tmp)
                return rb, reps

            def p3_groups(c, ko_list=(0, 1), accum=False, tail=False):
                """partial output projection psum-groups for chunk c over the
                given pj pairs; accum=True adds into DRAM instead of writing
                (used to split the last chunk's projection per pair)."""
                groups = []
                for tt in range(4):
                    for n2 in range(2):
                        def g(tt=tt, n2=n2):
                            mt = 4 * c + tt
                            prt = mmp.tile([128, CH], F32, tag="mm", name="mm")
                            for j, ko in enumerate(ko_list):
                                nc.tensor.matmul(
                                    prt,
                                    lhsT=pj_sb[c][ko][:, tt * 128:tt * 128 + 128],
                                    rhs=wo_sb[:, ko, n2 * CH:(n2 + 1) * CH],
                                    start=(j == 0),
                                    stop=(j == len(ko_list) - 1),
                                )
                            ob = ostg.tile([128, CH], F32, tag="ob", name="ob")
                            if tail and n2 == 0:
                                nc.scalar.copy(ob, prt)
                            else:
                                nc.vector.tensor_copy(ob, prt)
                            nc.scalar.dma_start(
                                out_v[:, mt, n2 * CH:(n2 + 1) * CH], ob,
                                accum_op=(mybir.AluOpType.add if accum
                                          else mybir.AluOpType.bypass))
                        groups.append(g)
                return groups

            pending = {}
            tks = {}
            lc = NCH - 1

            def prepass_groups():
                """ko=0 (pair 0) half of the last chunk's projection, stashed
                in SBUF tk tiles; runs inside the last attention block."""
                groups = []
                for tt in range(4):
                    for n2 in range(2):
                        def g(tt=tt, n2=n2):
                            prt = mmp.tile([128, CH], F32, tag="mm", name="mm")
                            nc.tensor.matmul(
                                prt,
                                lhsT=pj_sb[lc][0][:, tt * 128:tt * 128 + 128],
                                rhs=wo_sb[:, 0, n2 * CH:(n2 + 1) * CH],
                                start=True, stop=True,
                            )
                            tk = tkp.tile([128, CH], BF16, tag=f"tk{tt}{n2}",
                                          name=f"tk{tt}{n2}")
                            nc.vector.tensor_copy(tk, prt)
                            tks[(tt, n2)] = tk
                        groups.append(g)
                return groups

            for g in p1_groups(0):
                g()
            for c in range(NCH):
                inj0, inj1 = [], []
                if c > 0:
                    inj0.append(lambda c=c: fin_all(
                        c - 1, pending[(c - 1, 0)], pending[(c - 1, 1)]))
                if c + 1 < NCH:
                    p1 = p1_groups(c + 1)
                    inj0 += p1[:4]
                    inj1 += p1[4:]
                if c > 0:
                    inj1 += p3_groups(c - 1)
                r0 = attention(c, 0, inj0)
                if c == lc:
                    inj1.insert(0, lambda: fin_pair(lc, 0, *r0))
                    inj1 += prepass_groups()
                r1 = attention(c, 1, inj1, otfull=(c == lc))
                pending[(c, 0)] = r0
                pending[(c, 1)] = r1

            # ---- tail: finish pair 1, keep the PE warm through the chain
            # with dummy matmuls pegged to chain outputs, then the ko=1 half
            # of the projection + add of the stashed ko=0 half
            ots1, lb2_1 = pending[(lc, 1)]
            # normalize the last pair via PE transposes (PE is idle here and
            # they double as HAM warm-keepers): transpose out^T to put q on
            # partitions, where l becomes a per-partition column whose
            # reciprocal is ~free and the scale is a native tensor_scalar.
            # Full 128x128 transposes at base 0 only (walrus constraint);
            # the odd head is landed via one SBUF->SBUF shift DMA.
            for hh in (1, 0):
                otT = tkp.tile([128, 4, 65], F32, tag=f"otT{hh}",
                               name=f"otT{hh}")
                for j in range(4):
                    tp = mmp.tile([128, CH], F32, tag="mm", name="mm")
                    nc.tensor.transpose(
                        tp[:, :128], ots1[hh][:, j * 128:(j + 1) * 128],
                        idf[:])
                    nc.vector.tensor_copy(otT[:, j, :], tp[:, :65])
                recT = tkp.tile([128, 4], F32, tag=f"recT{hh}",
                                name=f"recT{hh}")
                nc.vector.reciprocal(recT, otT[:, :, 64])
                otn = tkp.tile([128, 4, 64], F32, tag=f"otn{hh}",
                               name=f"otn{hh}")
                shp = tkp.tile([64, CH], BF16, tag="shp", name="shp")
                for j in range(4):
                    nc.vector.tensor_scalar_mul(
                        otn[:, j, :], otT[:, j, :64], recT[:, j:j + 1])
                    tb = mmp.tile([128, CH], F32, tag="mm", name="mm")
                    nc.tensor.transpose(tb[0:64, :128], otn[:, j, :], idf[:])
                    if hh == 0:
                        nc.vector.tensor_copy(
                            pj_sb[lc][1][0:64, j * 128:(j + 1) * 128],
                            tb[0:64, :128])
                    else:
                        nc.vector.tensor_copy(
                            shp[:, j * 128:(j + 1) * 128], tb[0:64, :128])
                if hh == 1:
                    nc.sync.dma_start(pj_sb[lc][1][64:128, :], shp[:])
            for tt in range(4):
                for n2 in range(2):
                    prt = mmp.tile([128, CH], F32, tag="mm", name="mm")
                    nc.tensor.matmul(
                        prt,
                        lhsT=pj_sb[lc][1][:, tt * 128:tt * 128 + 128],
                        rhs=wo_sb[:, 1, n2 * CH:(n2 + 1) * CH],
                        start=True, stop=True,
                    )
                    ob = ostg.tile([128, CH], F32, tag="ob", name="ob")
                    nc.vector.tensor_add(ob, prt, tks[(tt, n2)][:])
                    eng = nc.scalar if (2 * tt + n2) % 2 == 0 else nc.sync
                    eng.dma_start(
                        out_v[:, 4 * lc + tt, n2 * CH:(n2 + 1) * CH], ob)

    nc.compile()
    _cached["nc"] = nc
    return nc


def make_in_maps(x, w_qkv, w_out):
    bf = ml_dtypes.bfloat16
    in_maps = []
    for core in range(N_CORES):
        b, h0 = core // 4, (core % 4) * HL
        c0 = h0 * DH
        in_maps.append({
            "xT": np.ascontiguousarray(x[b].T).astype(bf),
            "wq": w_qkv[:, c0:c0 + HL * DH].astype(bf),
            "wk": w_qkv[:, D + c0:D + c0 + HL * DH].astype(bf),
            "wv": w_qkv[:, 2 * D + c0:2 * D + c0 + HL * DH].astype(bf),
            "wo": w_out[c0:c0 + HL * DH, :].astype(bf),
        })
    return in_maps


def run_sharded(x, w_qkv, w_out, trace=False):
    nc = build()
    res = run_bass_kernel_spmd(
        nc, make_in_maps(x, w_qkv, w_out), core_ids=list(range(N_CORES)),
        trace=trace,
    )
    out = np.zeros((B, S, D), np.float32)
    for core in range(N_CORES):
        out[core // 4] += res.results[core]["out"]
    return out, res.exec_time_ns


def kernel(x, w_qkv, w_out):
    out, _ = run_sharded(x, w_qkv, w_out)
    return out
